# revision 49
# baseline (speedup 1.0000x reference)
"""Local (7x7 windowed) attention Trainium2 kernel — bf16 redesign.

Problem: B=1, N=4096 (T=4, H=W=32), C=384, 8 heads x hd=48, window 7x7
zero-padded (reference semantics: padded keys score exactly 0 -> weight
exp(0), value 0).

Sharding: data-parallel over positions. 8 cores; core c owns t-slice
c//2, query rows [16*(c%2), 16*(c%2)+16) (512 queries). Each core
recomputes k/v for a 3-row halo (24 rows = 768 halo positions,
zero-padded outside the image, matching the reference's zero padding).

Design (all matmul operands bf16, 1 cyc/col on PE at any N):
  1. qT = wq^T x^T (owned 512 cols only), kT = wk^T x^T (768 halo),
     v = x @ wv natural layout.
  2. vaug [128 pos, 6 jt, 8 h, 64]: every head block [1@0 | 0 | v(48)@16:64]
     -> denominator rows land on 32-aligned psum partitions 0/64 of the
     pair-packed O tile, and one copy per pt fills all 8 heads' values.
  3. S^T per (head, key-tile jt): single 64-contraction matmul into
     grouped psum tiles; exp on ACT (24 ops), 0/1 window mask applied as
     ONE fused DVE multiply per (pair, head) over the flat [128,1280] eT.
  4. O pair-packed [128, 512]: init matmul writes noob at partitions
     0/64 and zeros elsewhere; 12 accumulating matmuls (even heads ->
     partitions 0:64, odd -> 64:128).
  5. oT = bf16 copy of O; tail: ACT reciprocals of the den rows
     (oT[0]/oT[64], one act-table reload) interleaved with full-K sel2x
     broadcast matmuls -> B psum; nhat = oT * B (DVE) in bf16.
  6. proj: out[it] = sum_pr nhat[:,pr,it]^T @ wp[:,pr] + bias, DMA out.
"""

import os

import numpy as np
import ml_dtypes

import concourse.bacc as bacc
import concourse.mybir as mybir
import concourse.tile as tile
from concourse.bass_utils import run_bass_kernel_spmd

F = mybir.dt.float32
R = mybir.dt.float32r
BF = mybir.dt.bfloat16
E4 = mybir.dt.float8e4
BFNP = ml_dtypes.bfloat16
E4NP = ml_dtypes.float8_e4m3

NH = 8
HD = 48
WIN = 7
HALF = 3
T, HH, WW = 4, 32, 32
C = 384
NPOS = T * HH * WW
SCALE = HD ** -0.5

# per key-tile jt (4 halo rows): (query i_lo, span)
SPANS = {0: (0, 128), 1: (0, 256), 2: (64, 320), 3: (192, 320),
         4: (320, 192), 5: (448, 64)}
# exp groups: (psum col offset within group, jts), group width
GROUPS = [([(2, 0), (4, 320)], 512),
          ([(3, 0), (0, 320)], 448),
          ([(1, 0), (5, 256)], 320)]
# flat eT column offset per jt (group-major)
ET_OFF = {2: 0, 4: 320, 3: 512, 0: 832, 1: 960, 5: 1216}

_CACHE = {}
LAST_RESULT = None


def _act_recip(nc, out, in_):
    eng = nc.scalar
    ins = [eng.lower_ap(in_)]
    for val in (0.0, 1.0, 0.0):  # bias, scale, alpha
        ins.append(mybir.ImmediateValue(dtype=mybir.dt.float32, value=val))
    return eng.add_instruction(
        mybir.InstActivation(
            name=eng.bass.get_next_instruction_name(),
            func=mybir.ActivationFunctionType.Reciprocal,
            ins=ins,
            outs=[eng.lower_ap(out)],
        ))


def _build_nc():
    if "nc" in _CACHE:
        return _CACHE["nc"]
    nc = bacc.Bacc("TRN2", target_bir_lowering=False)

    d_xT = nc.dram_tensor("xT", [128, 3, 768], BF, kind="ExternalInput")
    d_wqk = nc.dram_tensor("wqk", [128, 3, 8, 128], BF, kind="ExternalInput")
    d_wv = nc.dram_tensor("wv", [128, 3, 384], BF, kind="ExternalInput")
    d_wp = nc.dram_tensor("wp", [128, 4, 384], BF, kind="ExternalInput")
    d_cst = nc.dram_tensor("cst", [1, 1152], BF, kind="ExternalInput")
    d_mask = nc.dram_tensor("mask", [128, 1280], BF, kind="ExternalInput")
    d_out = nc.dram_tensor("out", [512, 384], F, kind="ExternalOutput")

    EXP = mybir.ActivationFunctionType.Exp
    MUL = mybir.AluOpType.mult

    with tile.TileContext(nc) as tc:
        with tc.tile_pool(name="singles", bufs=1) as S:
            xT = S.tile([128, 3, 768], BF)
            wqk = S.tile([128, 3, 8, 128], BF)
            wv = S.tile([128, 3, 384], BF)
            wp = S.tile([128, 4, 384], BF)
            cst = S.tile([1, 1152], BF)
            mask = S.tile([128, 1280], BF)
            sel2x = S.tile([128, 128], BF)
            qT2 = S.tile([128, 4, 512], BF)
            kT2 = S.tile([128, 4, 768], BF)
            vaug = S.tile([128, 6, 8, 64], BF)
            nhat = S.tile([128, 4, 512], BF)
            recS = S.tile([128, 2048], BF)

            # input DMAs, ordered so the first matmuls can start early
            for k in range(3):
                nc.sync.dma_start(out=wqk[:, k], in_=d_wqk[:, k])
                nc.sync.dma_start(out=xT[:, k], in_=d_xT[:, k])
            nc.sync.dma_start(out=wv[:], in_=d_wv[:])
            nc.sync.dma_start(out=mask[:], in_=d_mask[:])
            nc.sync.dma_start(out=cst[:], in_=d_cst[:])
            nc.sync.dma_start(out=wp[:], in_=d_wp[:])

            # small consts packed into one DMA'd row
            noob = cst[0:1, 0:512]
            e2 = cst[0:1, 512:640]
            ones1 = cst[0:1, 640:768]
            bp = cst[0:1, 768:1152]

            # sel2x built on device (rows 0/64 select the den rows)
            nc.gpsimd.memset(sel2x[:], 0.0)
            nc.gpsimd.memset(sel2x[0:1, 0:64], 1.0)
            nc.gpsimd.memset(sel2x[64:65, 64:128], 1.0)
            nc.gpsimd.memset(vaug[:, :, :, 1:16], 0.0)
            nc.gpsimd.memset(vaug[:, :, :, 0:1], 1.0)
            # B matmul contracts over all 128 recS partitions; unused rows
            # must be finite zeros (0 * NaN garbage would poison PSUM)
            nc.gpsimd.memset(recS[:], 0.0)

            # ---- phase 1: q^T (owned only) / k^T (halo) / v ----------
            with tc.tile_pool(name="psA", bufs=2, space="PSUM") as psA:
                for pr in range(4):
                    qA = psA.tile([128, 512], F, tag="qA")
                    kA = psA.tile([128, 512], F, tag="kA")
                    kB = psA.tile([128, 256], F, tag="kB")
                    for k in range(3):
                        st, sp_ = (k == 0), (k == 2)
                        nc.tensor.matmul(qA[:], wqk[:, k, 2 * pr, :],
                                         xT[:, k, 96:608], start=st, stop=sp_)
                        nc.tensor.matmul(kA[:], wqk[:, k, 2 * pr + 1, :],
                                         xT[:, k, 0:512], start=st, stop=sp_)
                        nc.tensor.matmul(kB[:], wqk[:, k, 2 * pr + 1, :],
                                         xT[:, k, 512:768], start=st, stop=sp_)
                    nc.vector.tensor_copy(qT2[:, pr, :], qA[:])
                    nc.scalar.copy(kT2[:, pr, 0:512], kA[:])
                    nc.vector.tensor_copy(kT2[:, pr, 512:768], kB[:])
                for pt in range(6):
                    V = psA.tile([128, 384], F, tag="V")
                    for k in range(3):
                        nc.tensor.matmul(V[:], xT[:, k, 128 * pt:128 * pt + 128],
                                         wv[:, k, :], start=(k == 0), stop=(k == 2))
                    Vv = V[:].rearrange("p (h d) -> p h d", h=8)
                    nc.scalar.copy(vaug[:, pt, :, 16:64], Vv[:])

            # ---- phases 2-4 per head-pair ----------------------------
            with tc.tile_pool(name="psS", bufs=4, space="PSUM") as psS, \
                 tc.tile_pool(name="psO", bufs=2, space="PSUM") as psO, \
                 tc.tile_pool(name="psB", bufs=2, space="PSUM") as psB, \
                 tc.tile_pool(name="sbB", bufs=4) as sbB, \
                 tc.tile_pool(name="sb2", bufs=4) as sb2:
                oTs = []

                def emit_S(pr, e, eT):
                    for jts, gw in GROUPS:
                        G = psS.tile([128, 512], F, tag="G")
                        for jt, joff in jts:
                            ilo, spn = SPANS[jt]
                            nc.tensor.matmul(
                                G[:, joff:joff + spn],
                                kT2[64 * e:64 * e + 64, pr,
                                    128 * jt:128 * (jt + 1)],
                                qT2[64 * e:64 * e + 64, pr, ilo:ilo + spn],
                                start=True, stop=True, skip_group_check=True)
                        goff = ET_OFF[jts[0][0]]
                        nc.scalar.activation(eT[:, goff:goff + gw],
                                             G[:, 0:gw], EXP, scale=SCALE)
                    nc.vector.scalar_tensor_tensor(
                        out=eT[:], in0=eT[:], scalar=1.0, in1=mask[:],
                        op0=MUL, op1=MUL)

                def emit_recips(p2):
                    _act_recip(nc, recS[0:1, 512 * p2:512 * p2 + 512],
                               oTs[p2][0:1, :])
                    _act_recip(nc, recS[64:65, 512 * p2:512 * p2 + 512],
                               oTs[p2][64:65, :])

                def emit_bnhat(p2):
                    Bt = psB.tile([128, 512], F, tag="B")
                    nc.tensor.matmul(Bt[:], sel2x[:],
                                     recS[:, 512 * p2:512 * p2 + 512],
                                     start=True, stop=True)
                    nc.vector.tensor_mul(nhat[:, p2, :], oTs[p2][:], Bt[:])

                for pr in range(4):
                    eT0 = sb2.tile([128, 1280], BF, tag="eT")
                    eT1 = sb2.tile([128, 1280], BF, tag="eT")
                    emit_S(pr, 0, eT0)
                    O = psO.tile([128, 512], F, tag="O")
                    nc.tensor.matmul(O[:], e2, noob,
                                     start=True, stop=False,
                                     skip_group_check=True)
                    emit_S(pr, 1, eT1)
                    for e, eT in ((0, eT0), (1, eT1)):
                        for jt in range(6):
                            ilo, spn = SPANS[jt]
                            nc.tensor.matmul(
                                O[64 * e:64 * e + 64, ilo:ilo + spn],
                                vaug[:, jt, 2 * pr + e, :],
                                eT[:, ET_OFF[jt]:ET_OFF[jt] + spn],
                                start=False, stop=(e == 1 and jt == 5),
                                skip_group_check=True)
                    oT = sbB.tile([128, 512], BF, tag="oT")
                    nc.vector.tensor_copy(oT[:], O[:])
                    oTs.append(oT)
                # batched tail: ACT reciprocals (one act-table reload),
                # interleaved with the broadcast matmuls + nhat multiplies
                # so PE/DVE overlap the later reciprocal pairs
                for pr in range(4):
                    emit_recips(pr)
                    emit_bnhat(pr)

            # ---- phase 5: projection + bias --------------------------
            with tc.tile_pool(name="psP", bufs=4, space="PSUM") as psP, \
                 tc.tile_pool(name="sbo", bufs=4) as sbo:
                for it in range(4):
                    P = psP.tile([128, 384], F, tag="P")
                    for pr in range(4):
                        nc.tensor.matmul(P[:], nhat[:, pr, 128 * it:128 * (it + 1)],
                                         wp[:, pr, :], start=(pr == 0), stop=False)
                    nc.tensor.matmul(P[:], ones1, bp,
                                     start=False, stop=True)
                    ot = sbo.tile([128, 384], F, tag="ot")
                    nc.scalar.copy(ot[:], P[:])
                    nc.sync.dma_start(out=d_out[128 * it:128 * (it + 1), :],
                                      in_=ot[:])

    nc.compile()
    _CACHE["nc"] = nc
    return nc


def _host_consts(w_qkv, w_proj, b_proj):
    wqk = np.zeros((128, 3, 8, 128), np.float32)
    for k in range(3):
        rows = slice(k * 128, (k + 1) * 128)
        for pr in range(4):
            for s in range(2):  # 0 = q block, 1 = k block
                off = 384 * s
                wqk[:, k, 2 * pr + s, 0:48] = \
                    w_qkv[rows, off + 48 * (2 * pr):off + 48 * (2 * pr) + 48]
                wqk[:, k, 2 * pr + s, 64:112] = \
                    w_qkv[rows, off + 48 * (2 * pr + 1):off + 48 * (2 * pr + 1) + 48]
    wvp = np.ascontiguousarray(np.transpose(
        w_qkv[:, 768:1152].reshape(3, 128, 384), (1, 0, 2)))
    # proj weights: partitions 0:48 even-head rows, 65:113 odd-head rows
    wp = np.zeros((128, 4, 384), np.float32)
    for pr in range(4):
        wp[16:64, pr, :] = w_proj[96 * pr:96 * pr + 48, :]
        wp[80:128, pr, :] = w_proj[96 * pr + 48:96 * pr + 96, :]

    # 0/1 window mask over the flat eT layout
    mask = np.zeros((128, 1280), np.float32)
    kk = np.arange(128)
    for jt, (ilo, spn) in SPANS.items():
        off = ET_OFF[jt]
        kr = 4 * jt + kk // 32 - 3
        kx = kk % 32
        qq = ilo + np.arange(spn)
        qr = qq // 32
        qx = qq % 32
        valid = ((np.abs(qr[None, :] - kr[:, None]) <= 3)
                 & (np.abs(qx[None, :] - kx[:, None]) <= 3))
        mask[:, off:off + spn] = valid

    cst = np.zeros((1, 1152), np.float32)
    for qy in range(16):
        for qx in range(32):
            cst[0, 32 * qy + qx] = 7.0 * (max(0, 3 - qx) + max(0, qx - 28))
    cst[0, 512] = 1.0    # e2 one-hot at 0
    cst[0, 576] = 1.0    # e2 one-hot at 64
    cst[0, 640:768] = 1.0  # ones1
    cst[0, 768:1152] = b_proj
    return dict(wqk=wqk.astype(BFNP), wv=wvp.astype(BFNP),
                wp=wp.astype(BFNP), mask=mask.astype(BFNP),
                cst=cst.astype(BFNP))


def kernel(x, w_qkv, w_proj, b_proj, H=32, W=32):
    global LAST_RESULT
    x = np.asarray(x, np.float32)
    w_qkv = np.asarray(w_qkv, np.float32)
    w_proj = np.asarray(w_proj, np.float32)
    b_proj = np.asarray(b_proj, np.float32)
    assert x.shape == (1, NPOS, C) and int(H) == 32 and int(W) == 32

    nc = _build_nc()
    consts = _host_consts(w_qkv, w_proj, b_proj)

    x4 = x[0].reshape(T, HH, WW, C)
    in_maps = []
    for c in range(8):
        t, ry0 = c // 2, 16 * (c % 2)
        xh = np.zeros((24, WW, C), np.float32)
        lo, hi = ry0 - 3, ry0 + 21
        slo, shi = max(lo, 0), min(hi, HH)
        xh[slo - lo:shi - lo] = x4[t, slo:shi]
        xT = np.ascontiguousarray(
            xh.reshape(768, C).T.reshape(3, 128, 768).transpose(1, 0, 2)
        ).astype(BFNP)
        in_maps.append({"xT": xT, **consts})

    trace = bool(int(os.environ.get("TRACE", "0")))
    res = run_bass_kernel_spmd(nc, in_maps, core_ids=list(range(8)),
                               trace=trace)
    LAST_RESULT = res
    out = np.concatenate([res.results[c]["out"] for c in range(8)], axis=0)
    return out.reshape(1, NPOS, C)
